# revision 8
# baseline (speedup 1.0000x reference)
"""Trainium2 Bass kernel for nn_ContrastCELoss (seg NLL + SupCon-style contrastive loss).

Self-contained: hardcodes shapes for
  feats   [2, 128, 64, 2048] f32
  outputs [2,  20, 64, 2048] f32
  labels  [2,      64, 2048] int
  loss_w  [20]               f32
Returns scalar f32:  seg_loss + 0.1 * contrast_loss.

Sharding: data-parallel over the 262144 pixels for the NLL phase (32768/core);
the sampled contrastive matrix (M<=1024 anchors) is row-sharded 128 rows/core
with the anchor matrix replicated. Scalar partials are combined on host.

The index sampling (_sample_indices) is host-side in the original module too
(numpy RNG); predict=argmax(outputs) feeds only that sampling and is computed
on host with identical first-max semantics.
"""

import math
from contextlib import ExitStack

import numpy as np

import concourse.bass as bass
import concourse.bacc as bacc
import concourse.tile as tile
from concourse import mybir
from concourse.bass_utils import run_bass_kernel_spmd

# ---- problem constants ----
B, D, H, W, C = 2, 128, 64, 2048, 20
NPIX = B * H * W                     # 262144
NCORES = 8
PIX = NPIX // NCORES                 # 32768 pixels per core
Q = 6                                # row-groups per class -> 20*6=120 partitions
L = (PIX + Q - 1) // Q               # 5462 pixels per partition
LPAD = Q * L                         # 32772 (padded pixel count per core)
NP_NLL = C * Q                       # 120 partitions used in NLL phase

MP = 1024                            # padded anchor count (M = T*n_view <= 1024)
RB = MP // NCORES                    # 128 anchor rows per core

MAX_SAMPLES = 1024
MAX_VIEWS = 100
TEMP = 0.07
BASE_TEMP = 0.07
IGNORE = (0,)
LOSS_WEIGHT = 0.1

F32 = mybir.dt.float32
I8 = mybir.dt.int8


def _sample_indices(labels, predict, seed=0):
    """Host-side replica of _hard_anchor_sampling index selection (verbatim
    port of the original module's numpy logic, RandomState(seed) sequence)."""
    rng = np.random.RandomState(seed)
    Bn = labels.shape[0]
    classes, total = [], 0
    for ii in range(Bn):
        cs = [int(c) for c in np.unique(labels[ii])
              if int(c) not in IGNORE and int((labels[ii] == c).sum()) > MAX_VIEWS]
        classes.append(cs)
        total += len(cs)
    assert total > 0
    n_view = min(MAX_SAMPLES // total, MAX_VIEWS)
    b_idx = np.zeros((total, n_view), np.int64)
    p_idx = np.zeros((total, n_view), np.int64)
    y_ = np.zeros((total,), np.float32)
    ptr = 0
    for ii in range(Bn):
        for c in classes[ii]:
            hard = np.nonzero((labels[ii] == c) & (predict[ii] != c))[0]
            easy = np.nonzero((labels[ii] == c) & (predict[ii] == c))[0]
            nh, ne = len(hard), len(easy)
            if nh >= n_view / 2 and ne >= n_view / 2:
                kh = n_view // 2
                ke = n_view - kh
            elif nh >= n_view / 2:
                ke = ne
                kh = n_view - ke
            else:
                kh = nh
                ke = n_view - kh
            hard = hard[rng.permutation(nh)[:kh]]
            easy = easy[rng.permutation(ne)[:ke]]
            idx = np.concatenate([hard, easy])
            b_idx[ptr] = ii
            p_idx[ptr] = idx
            y_[ptr] = c
            ptr += 1
    return b_idx, p_idx, y_, n_view


_CACHED_NC = None


def _build_bass():
    """One SPMD program for all 8 cores (per-core data differs via in_maps)."""
    global _CACHED_NC
    if _CACHED_NC is not None:
        return _CACHED_NC

    nc = bacc.Bacc("TRN2", target_bir_lowering=False, debug=False,
                   num_devices=NCORES)

    xcls = nc.dram_tensor("xcls", [NP_NLL, L], F32, kind="ExternalInput").ap()
    oh = nc.dram_tensor("oh", [NP_NLL, L], I8, kind="ExternalInput").ap()
    featT = nc.dram_tensor("featT", [128, MP], F32, kind="ExternalInput").ap()
    featRows = nc.dram_tensor("featRows", [128, RB], F32, kind="ExternalInput").ap()
    yceq = nc.dram_tensor("yceq", [128, MP], I8, kind="ExternalInput").ap()
    ycne = nc.dram_tensor("ycne", [128, MP], I8, kind="ExternalInput").ap()
    pc = nc.dram_tensor("pc", [128, 4], F32, kind="ExternalInput").ap()
    res = nc.dram_tensor("res", [128, 4], F32, kind="ExternalOutput").ap()

    with tile.TileContext(nc) as tc, ExitStack() as ctx:
        sb = ctx.enter_context(tc.tile_pool(name="sb", bufs=1))
        ps = ctx.enter_context(tc.tile_pool(name="ps", bufs=1, space="PSUM"))

        # ---------- NLL phase: s1[p] = sum_j onehot[p,j] * outputs[p,j] ----------
        x_t = sb.tile([NP_NLL, L], F32)
        nc.sync.dma_start(x_t[:, :], xcls[:, :])
        oh_t = sb.tile([NP_NLL, L], I8)
        nc.sync.dma_start(oh_t[:, :], oh[:, :])

        junk1 = sb.tile([NP_NLL, L], F32)
        s1 = sb.tile([128, 1], F32)
        nc.vector.memset(s1[:, :], 0.0)
        nc.vector.scalar_tensor_tensor(
            out=junk1[:, :],
            in0=oh_t[:, :],
            scalar=0.0,
            in1=x_t[:, :],
            op0=mybir.AluOpType.bypass,
            op1=mybir.AluOpType.mult,
            accum_out=s1[0:NP_NLL, :],
        )

        # ---------- contrastive phase ----------
        ft = sb.tile([128, MP], F32)
        nc.sync.dma_start(ft[:, :], featT[:, :])
        fr = sb.tile([128, RB], F32)
        nc.sync.dma_start(fr[:, :], featRows[:, :])
        yq = sb.tile([128, MP], I8)
        nc.sync.dma_start(yq[:, :], yceq[:, :])
        yn = sb.tile([128, MP], I8)
        nc.sync.dma_start(yn[:, :], ycne[:, :])
        pct = sb.tile([128, 4], F32)
        nc.sync.dma_start(pct[:, :], pc[:, :])

        raw = ps.tile([128, MP], F32)
        nc.tensor.matmul(raw[:, 0:512], fr[:, :], ft[:, 0:512], start=True, stop=True)
        nc.tensor.matmul(raw[:, 512:1024], fr[:, :], ft[:, 512:1024], start=True, stop=True)

        rmax = sb.tile([128, 1], F32)
        nc.vector.reduce_max(rmax[:, :], raw[:, :], axis=mybir.AxisListType.X)
        nrmax = sb.tile([128, 1], F32)
        nc.vector.tensor_scalar_mul(nrmax[:, :], rmax[:, :], -1.0)

        # exp_l = exp(raw - rmax)   (off-diagonal terms may underflow to exactly 0)
        expl = sb.tile([128, MP], F32)
        nc.scalar.activation(expl[:, :], raw[:, :],
                             mybir.ActivationFunctionType.Exp,
                             bias=nrmax[:, :], scale=1.0)

        # ns = sum_j [y_col[j] != y_row[p]] * exp_l   (diag + same-class excluded)
        junk2 = sb.tile([128, MP], F32)
        ns = sb.tile([128, 1], F32)
        nc.vector.scalar_tensor_tensor(
            out=junk2[:, :], in0=yn[:, :], scalar=pct[:, 0:1], in1=expl[:, :],
            op0=mybir.AluOpType.not_equal, op1=mybir.AluOpType.mult,
            accum_out=ns[:, :])

        # logden = ln(exp_l + ns)
        logden = sb.tile([128, MP], F32)
        nc.scalar.activation(logden[:, :], expl[:, :],
                             mybir.ActivationFunctionType.Ln,
                             bias=ns[:, :], scale=1.0)

        # pld = sum_j [poisoned y_col == y_row] * logden   (positives excl diag)
        junk3 = sb.tile([128, MP], F32)
        pld = sb.tile([128, 1], F32)
        nc.vector.scalar_tensor_tensor(
            out=junk3[:, :], in0=yq[:, :], scalar=pct[:, 0:1], in1=logden[:, :],
            op0=mybir.AluOpType.is_equal, op1=mybir.AluOpType.mult,
            accum_out=pld[:, :])

        # praw = sum_j [poisoned y_col == y_row] * raw
        junk4 = sb.tile([128, MP], F32)
        praw = sb.tile([128, 1], F32)
        nc.vector.scalar_tensor_tensor(
            out=junk4[:, :], in0=yq[:, :], scalar=pct[:, 0:1], in1=raw[:, :],
            op0=mybir.AluOpType.is_equal, op1=mybir.AluOpType.mult,
            accum_out=praw[:, :])

        # mlpp = (praw - pld) * inv_cnt - rmax
        t1 = sb.tile([128, 1], F32)
        nc.vector.tensor_sub(t1[:, :], praw[:, :], pld[:, :])
        t2 = sb.tile([128, 1], F32)
        nc.vector.tensor_mul(t2[:, :], t1[:, :], pct[:, 1:2])
        mlpp = sb.tile([128, 1], F32)
        nc.vector.tensor_sub(mlpp[:, :], t2[:, :], rmax[:, :])

        out_sb = sb.tile([128, 4], F32)
        nc.vector.memset(out_sb[:, :], 0.0)
        nc.vector.tensor_copy(out_sb[:, 0:1], s1[:, :])
        nc.vector.tensor_copy(out_sb[:, 1:2], mlpp[:, :])
        nc.vector.tensor_copy(out_sb[:, 2:3], ns[:, :])
        nc.vector.tensor_copy(out_sb[:, 3:4], rmax[:, :])
        nc.sync.dma_start(res[:, :], out_sb[:, :])

    nc.compile()
    _CACHED_NC = nc
    return nc


def _prepare_in_maps(outputs, labels, feats, loss_w):
    """Host-side sharding + aux tensor construction. Returns (in_maps, host_ctx)."""
    outputs = np.ascontiguousarray(np.asarray(outputs, dtype=np.float32))
    feats = np.asarray(feats, dtype=np.float32)
    labels = np.asarray(labels)
    loss_w = np.asarray(loss_w, dtype=np.float32)

    lab_flat = labels.reshape(B, H * W).astype(np.int64)
    out_flat = outputs.reshape(B, C, H * W)

    # predict: identical first-max semantics to jnp.argmax over class axis
    predict = out_flat.argmax(axis=1)

    b_idx, p_idx, y_, n_view = _sample_indices(lab_flat, predict)
    T = y_.shape[0]
    M = T * n_view
    assert M <= MP, (T, n_view)

    # gather anchors: X_[t, v, :] = feats[b, :, pix]; view-major row order
    ff = feats.reshape(B, D, H * W)
    X_ = ff[b_idx, :, p_idx]                     # [T, n_view, D]
    feat = np.transpose(X_, (1, 0, 2)).reshape(M, D)

    s = np.float32(1.0 / math.sqrt(TEMP))
    featT = np.zeros((D, MP), np.float32)
    featT[:, :M] = (feat * s).T

    y_full = np.full((MP,), -1.0, np.float32)
    y_full[:M] = np.tile(y_, n_view)             # y of anchor m is y_[m % T]

    # positives count per anchor (same class, excluding self)
    cnt_cls = {c: int((y_ == c).sum()) * n_view for c in np.unique(y_)}
    inv_cnt_full = np.zeros((MP,), np.float32)
    for m in range(M):
        inv_cnt_full[m] = 1.0 / (cnt_cls[y_full[m]] - 1)

    ycne_row = np.clip(y_full, -128, 127).astype(np.int8)     # -1 pad, 0..19

    # NLL host prep: pad pixel dim, build per-core onehot (int8)
    lab_all = lab_flat.reshape(-1)               # [262144]
    cls_of_part = np.repeat(np.arange(C, dtype=np.int64), Q)  # [120]

    # outputs per core: pixel index g = b*H*W + h*W + w; core k owns [k*PIX,(k+1)*PIX)
    out_bc = out_flat.transpose(1, 0, 2).reshape(C, B * H * W)  # [C, NPIX]
    in_maps = [None] * NCORES
    for k in range(NCORES):
        sl = slice(k * PIX, (k + 1) * PIX)
        xp = np.zeros((C, LPAD), np.float32)
        xp[:, :PIX] = out_bc[:, sl]
        xcls_k = xp.reshape(NP_NLL, L)

        labp = np.full((LPAD,), -1, np.int64)
        labp[:PIX] = lab_all[sl]
        lab_k = labp.reshape(Q, L)               # [6, 5462]
        # onehot [120, 5462]: partition p=(c,q) -> labels row q == class c
        oh_k = (lab_k[None, :, :] == np.arange(C, dtype=np.int64)[:, None, None])
        oh_k = oh_k.reshape(NP_NLL, L).astype(np.int8)

        rows = slice(k * RB, (k + 1) * RB)
        y_row = y_full[rows].copy()
        y_row[y_row < 0] = -2.0                  # pad rows: -2 (never matches -1 pads)
        inv_cnt = inv_cnt_full[rows]

        yceq_k = np.broadcast_to(ycne_row, (128, MP)).copy()
        for p in range(RB):
            yceq_k[p, k * RB + p] = -9           # poison diagonal -> excluded
        # ycne: pad cols equal own-row y so not_equal excludes them from neg-sum
        ycne_k = np.broadcast_to(ycne_row, (128, MP)).copy()
        ycne_k[:, M:] = y_row[:, None].astype(np.int8)

        pc_k = np.zeros((128, 4), np.float32)
        pc_k[:, 0] = y_row
        pc_k[:, 1] = inv_cnt

        fr_k = np.ascontiguousarray(featT[:, rows])

        in_maps[k] = {
            "xcls": xcls_k,
            "oh": oh_k,
            "featT": featT,
            "featRows": fr_k,
            "yceq": yceq_k,
            "ycne": ycne_k,
            "pc": pc_k,
        }

    host_ctx = {
        "loss_w": loss_w,
        "lab_all": lab_all,
        "cls_of_part": cls_of_part,
        "M": M,
        "n_view": n_view,
    }
    return in_maps, host_ctx


def _combine(results, host_ctx):
    loss_w = host_ctx["loss_w"]
    lab_all = host_ctx["lab_all"]
    cls_of_part = host_ctx["cls_of_part"]
    M = host_ctx["M"]

    # seg loss: -(sum_p lw[class(p)] * s1[p]) / sum(lw[labels])
    wlogp = 0.0
    for k in range(NCORES):
        s1 = results[k]["res"][:NP_NLL, 0].astype(np.float64)
        wlogp += float((loss_w[cls_of_part].astype(np.float64) * s1).sum())
    hist = np.bincount(lab_all, minlength=C).astype(np.float64)
    wsum = float((hist * loss_w.astype(np.float64)).sum())
    seg = -wlogp / wsum

    # contrastive: row is NaN iff its neg-sum is exactly 0 (reference's 0*inf)
    mlpp = np.concatenate([results[k]["res"][:, 1] for k in range(NCORES)])
    nsv = np.concatenate([results[k]["res"][:, 2] for k in range(NCORES)])
    mlpp = mlpp[:M].astype(np.float64)
    nsv = nsv[:M]
    mlpp[nsv == 0.0] = np.nan
    contrast = -(TEMP / BASE_TEMP) * (mlpp.mean())

    return np.float32(seg + LOSS_WEIGHT * contrast)


def _run(inputs, trace=False, trace_kwargs=None):
    nc = _build_bass()
    in_maps, host_ctx = _prepare_in_maps(
        inputs["outputs"], inputs["labels"], inputs["feats"], inputs["loss_w"])
    bkr = run_bass_kernel_spmd(nc, in_maps, list(range(NCORES)), trace=trace,
                               **(trace_kwargs or {}))
    val = _combine(bkr.results, host_ctx)
    return val, bkr


def kernel(feats, outputs, labels, loss_w):
    val, _ = _run({"feats": feats, "outputs": outputs,
                   "labels": labels, "loss_w": loss_w})
    return np.array(val, dtype=np.float32)
